# revision 1
# baseline (speedup 1.0000x reference)
"""Trainium2 Bass kernel for graph_coarsening_vn (nn_ASH_DGT_31791347925004).

Pipeline (8 NeuronCores, SPMD):
  Phase A (HW): inter = A_bin @ A_bin (A symmetric) — 2D-sharded: core c=(i,j)
      computes inter[512i:512i+512, 1024j:1024j+1024] in bf16 (exact ints).
  Host: jaccard = inter/union (IEEE f32), stable-descending pair order,
      union-find == connected components of the minimal prefix with
      NUM_SUPER components, labels by first occurrence, P = S D^{-1/2}.
  Phase B (HW): partial S^T A S and S^T X sharded over node axis, one core
      per 256-row block; host sums partials and applies D^{-1/2} scaling.
"""
import os
import sys

for _p in ("/opt/trn_rl_repo", "/root/.axon_site/_ro/trn_rl_repo"):
    if os.path.isdir(_p) and _p not in sys.path:
        sys.path.append(_p)

import numpy as np
import ml_dtypes

N = 2048
D = 512
NUM_SUPER = 512
EPS = 1e-10
NCORES = 8
KT = 16
BLK = 256

_CACHE = {}


# ---------------------------------------------------------------- bass builders
def _build_phase_a():
    import concourse.bacc as bacc
    import concourse.tile as tile
    from concourse import mybir

    nc = bacc.Bacc("TRN2", target_bir_lowering=False, debug=False, num_devices=NCORES)
    BF = mybir.dt.bfloat16
    F32 = mybir.dt.float32
    a_lhs = nc.dram_tensor("a_lhs", [N, 512], BF, kind="ExternalInput")
    a_rhs = nc.dram_tensor("a_rhs", [N, 1024], BF, kind="ExternalInput")
    out = nc.dram_tensor("inter_blk", [512, 1024], BF, kind="ExternalOutput")

    with tile.TileContext(nc) as tc:
        with (
            tc.tile_pool(name="lhs", bufs=1) as lhs_pool,
            tc.tile_pool(name="rhs", bufs=1) as rhs_pool,
            tc.tile_pool(name="psum", bufs=1, space="PSUM") as psum_pool,
            tc.tile_pool(name="osb", bufs=1) as osb_pool,
        ):
            lhs_t, rhs_t = [], []
            for kt in range(KT):
                lt = lhs_pool.tile([128, 512], BF, name=f"lhs_{kt}")
                nc.sync.dma_start(lt[:], a_lhs[kt * 128 : (kt + 1) * 128, :])
                lhs_t.append(lt)
                rt = rhs_pool.tile([128, 1024], BF, name=f"rhs_{kt}")
                nc.sync.dma_start(rt[:], a_rhs[kt * 128 : (kt + 1) * 128, :])
                rhs_t.append(rt)
            psums = [
                [
                    psum_pool.tile([128, 512], F32, name=f"ps_{m}_{nb}")
                    for nb in range(2)
                ]
                for m in range(4)
            ]
            for kt in range(KT):
                for m in range(4):
                    for nb in range(2):
                        nc.tensor.matmul(
                            psums[m][nb][:],
                            lhs_t[kt][:, m * 128 : (m + 1) * 128],
                            rhs_t[kt][:, nb * 512 : (nb + 1) * 512],
                            start=(kt == 0),
                            stop=(kt == KT - 1),
                        )
            for m in range(4):
                for nb in range(2):
                    ot = osb_pool.tile([128, 512], BF, name=f"osb_{m}_{nb}")
                    nc.vector.tensor_copy(ot[:], psums[m][nb][:])
                    nc.sync.dma_start(
                        out[m * 128 : (m + 1) * 128, nb * 512 : (nb + 1) * 512],
                        ot[:],
                    )
    nc.compile()
    return nc


def _build_phase_b():
    import concourse.bacc as bacc
    import concourse.tile as tile
    from concourse import mybir

    M = NUM_SUPER
    nc = bacc.Bacc("TRN2", target_bir_lowering=False, debug=False, num_devices=NCORES)
    BF = mybir.dt.bfloat16
    F32 = mybir.dt.float32
    a_cols = nc.dram_tensor("a_cols", [N, BLK], BF, kind="ExternalInput")
    s_full = nc.dram_tensor("s_full", [N, M], BF, kind="ExternalInput")
    s_rows = nc.dram_tensor("s_rows", [BLK, M], BF, kind="ExternalInput")
    x_rows = nc.dram_tensor("x_rows", [BLK, M], BF, kind="ExternalInput")
    pa_raw = nc.dram_tensor("pa_raw", [M, M], F32, kind="ExternalOutput")
    px_raw = nc.dram_tensor("px_raw", [M, M], F32, kind="ExternalOutput")

    with tile.TileContext(nc) as tc:
        with (
            tc.tile_pool(name="insb", bufs=1) as insb,
            tc.tile_pool(name="psum", bufs=8, space="PSUM") as psum_pool,
            tc.tile_pool(name="osb", bufs=1) as osb,
        ):
            ac_t, sf_t = [], []
            for kt in range(KT):
                act = insb.tile([128, BLK], BF, name=f"ac_{kt}")
                nc.sync.dma_start(act[:], a_cols[kt * 128 : (kt + 1) * 128, :])
                ac_t.append(act)
                sft = insb.tile([128, M], BF, name=f"sf_{kt}")
                nc.sync.dma_start(sft[:], s_full[kt * 128 : (kt + 1) * 128, :])
                sf_t.append(sft)
            sr_t, xr_t = [], []
            for q in range(2):
                srt = insb.tile([128, M], BF, name=f"sr_{q}")
                nc.sync.dma_start(srt[:], s_rows[q * 128 : (q + 1) * 128, :])
                sr_t.append(srt)
                xrt = insb.tile([128, M], BF, name=f"xr_{q}")
                nc.sync.dma_start(xrt[:], x_rows[q * 128 : (q + 1) * 128, :])
                xr_t.append(xrt)

            t1_ps = [
                psum_pool.tile([128, M], F32, name=f"t1ps_{mi}", tag="ps")
                for mi in range(2)
            ]
            for kt in range(KT):
                for mi in range(2):
                    nc.tensor.matmul(
                        t1_ps[mi][:],
                        ac_t[kt][:, mi * 128 : (mi + 1) * 128],
                        sf_t[kt][:],
                        start=(kt == 0),
                        stop=(kt == KT - 1),
                    )
            t1_sb = []
            for mi in range(2):
                t1s = osb.tile([128, M], BF, name=f"t1sb_{mi}")
                nc.vector.tensor_copy(t1s[:], t1_ps[mi][:])
                t1_sb.append(t1s)

            for m in range(4):
                pap = psum_pool.tile([128, M], F32, name=f"paps_{m}", tag="ps")
                for q in range(2):
                    nc.tensor.matmul(
                        pap[:],
                        sr_t[q][:, m * 128 : (m + 1) * 128],
                        t1_sb[q][:],
                        start=(q == 0),
                        stop=(q == 1),
                    )
                pao = osb.tile([128, M], F32, name=f"pao_{m}")
                nc.vector.tensor_copy(pao[:], pap[:])
                nc.sync.dma_start(pa_raw[m * 128 : (m + 1) * 128, :], pao[:])

            for m in range(4):
                pxp = psum_pool.tile([128, M], F32, name=f"pxps_{m}", tag="ps")
                for q in range(2):
                    nc.tensor.matmul(
                        pxp[:],
                        sr_t[q][:, m * 128 : (m + 1) * 128],
                        xr_t[q][:],
                        start=(q == 0),
                        stop=(q == 1),
                    )
                pxo = osb.tile([128, M], F32, name=f"pxo_{m}")
                nc.vector.tensor_copy(pxo[:], pxp[:])
                nc.sync.dma_start(px_raw[m * 128 : (m + 1) * 128, :], pxo[:])
    nc.compile()
    return nc


def _get_nc(which):
    if which not in _CACHE:
        _CACHE[which] = _build_phase_a() if which == "a" else _build_phase_b()
    return _CACHE[which]


# ---------------------------------------------------------------- host algorithm
def _connected_components(n, ei, ej):
    """Connected components of an undirected graph. Returns (ncomp, labels).
    Uses scipy if available, else numpy label propagation with pointer jumping."""
    try:
        from scipy.sparse import coo_matrix
        from scipy.sparse.csgraph import connected_components

        g = coo_matrix((np.ones(ei.size, np.int8), (ei, ej)), shape=(n, n))
        return connected_components(g, directed=False)
    except ImportError:
        lab = np.arange(n, dtype=np.int64)
        while True:
            m1 = np.minimum(lab[ei], lab[ej])
            nxt = lab.copy()
            np.minimum.at(nxt, ei, m1)
            np.minimum.at(nxt, ej, m1)
            # pointer jumping
            for _ in range(40):
                nxt2 = nxt[nxt]
                if np.array_equal(nxt2, nxt):
                    break
                nxt = nxt2
            if np.array_equal(nxt, lab):
                break
            lab = nxt
        uniq, lab2 = np.unique(lab, return_inverse=True)
        return uniq.size, lab2


def _labels_from_inter(inter_f32, deg_f32):
    n = N
    iu, ju = np.triu_indices(n, k=1)
    iu = iu.astype(np.int32)
    ju = ju.astype(np.int32)
    inter_t = inter_f32[iu, ju]
    union = deg_f32[iu] + deg_f32[ju] - inter_t
    union = np.where(union == 0, np.float32(EPS), union).astype(np.float32)
    sims = (inter_t.astype(np.float32) / union).astype(np.float32)

    target_comp = NUM_SUPER
    m_total = sims.shape[0]
    T = 1 << 16
    order = None
    while True:
        if T >= m_total:
            order = np.argsort(-sims, kind="stable")
            break
        v = np.partition(sims, m_total - T)[m_total - T]
        cand = np.flatnonzero(sims >= v)
        ncomp, _ = _connected_components(n, iu[cand], ju[cand])
        if ncomp <= target_comp:
            sub = np.argsort(-sims[cand], kind="stable")
            order = cand[sub]
            break
        T <<= 2

    oi = iu[order]
    oj = ju[order]

    def ncomp_of(t):
        return _connected_components(n, oi[:t], oj[:t])[0]

    lo, hi = 0, 1024
    while True:
        if hi >= order.size:
            hi = order.size
            break
        if ncomp_of(hi) <= target_comp:
            break
        lo = hi
        hi <<= 2
    while hi - lo > 1:
        mid = (lo + hi) // 2
        if ncomp_of(mid) <= target_comp:
            hi = mid
        else:
            lo = mid
    t_star = hi
    ncomp, raw = _connected_components(n, oi[:t_star], oj[:t_star])
    assert ncomp == target_comp, (ncomp, target_comp)

    first_idx = np.full(ncomp, n, dtype=np.int64)
    np.minimum.at(first_idx, raw, np.arange(n))
    rank = np.empty(ncomp, dtype=np.int32)
    rank[np.argsort(first_idx)] = np.arange(ncomp, dtype=np.int32)
    return rank[raw]


# ---------------------------------------------------------------- main entry
def kernel(X, A):
    from concourse.bass_utils import run_bass_kernel_spmd

    X = np.asarray(X, dtype=np.float32)
    A = np.asarray(A, dtype=np.float32)
    A_bin = (A > 0).astype(np.float32)
    A_bf16 = A_bin.astype(ml_dtypes.bfloat16)

    # ---- Phase A: inter = A_bin @ A_bin
    nc_a = _get_nc("a")
    in_maps = []
    for c in range(NCORES):
        i, j = c // 2, c % 2
        in_maps.append(
            {
                "a_lhs": np.ascontiguousarray(A_bf16[:, 512 * i : 512 * (i + 1)]),
                "a_rhs": np.ascontiguousarray(A_bf16[:, 1024 * j : 1024 * (j + 1)]),
            }
        )
    res_a = run_bass_kernel_spmd(nc_a, in_maps, core_ids=list(range(NCORES)))
    inter = np.empty((N, N), dtype=np.float32)
    for c in range(NCORES):
        i, j = c // 2, c % 2
        inter[512 * i : 512 * (i + 1), 1024 * j : 1024 * (j + 1)] = res_a.results[
            c
        ]["inter_blk"].astype(np.float32)

    deg = inter.diagonal().copy()  # exact: diag(A@A) = row degrees for 0/1 A

    # ---- Host: labels + P
    labels = _labels_from_inter(inter, deg)
    counts = np.bincount(labels, minlength=NUM_SUPER).astype(np.float32)
    scale = (np.float32(1.0) / np.sqrt(counts + np.float32(EPS))).astype(np.float32)
    P = np.zeros((N, NUM_SUPER), dtype=np.float32)
    P[np.arange(N), labels] = scale[labels]

    # ---- Phase B: partial S^T A S and S^T X
    S_bf16 = np.zeros((N, NUM_SUPER), dtype=ml_dtypes.bfloat16)
    S_bf16[np.arange(N), labels] = 1.0
    X_bf16 = X.astype(ml_dtypes.bfloat16)
    nc_b = _get_nc("b")
    in_maps_b = []
    for c in range(NCORES):
        lo, hi = BLK * c, BLK * (c + 1)
        in_maps_b.append(
            {
                "a_cols": np.ascontiguousarray(A_bf16[:, lo:hi]),
                "s_full": S_bf16,
                "s_rows": np.ascontiguousarray(S_bf16[lo:hi, :]),
                "x_rows": np.ascontiguousarray(X_bf16[lo:hi, :]),
            }
        )
    res_b = run_bass_kernel_spmd(nc_b, in_maps_b, core_ids=list(range(NCORES)))
    pa = np.zeros((NUM_SUPER, NUM_SUPER), np.float64)
    px = np.zeros((NUM_SUPER, D), np.float64)
    for c in range(NCORES):
        pa += res_b.results[c]["pa_raw"].astype(np.float64)
        px += res_b.results[c]["px_raw"].astype(np.float64)

    A_coarse = (
        scale[:, None] * scale[None, :] * pa.astype(np.float32)
    ).astype(np.float32)
    X_coarse = (scale[:, None] * px.astype(np.float32)).astype(np.float32)
    return (X_coarse, A_coarse, P)
